# revision 101
# baseline (speedup 1.0000x reference)
"""GPT-2-style causal attention block on 8 TRN2 NeuronCores (Bass/Tile).

Sharding (Megatron-style, per the hint): core c handles batch b = c // 4 and
head-group g = c % 4 (4 of the 16 heads).  Each core computes, fully locally:
  QKV projection (its 4 heads' columns), causal softmax attention for its
  4 heads, and the row-sharded output projection partial [S, D].
The host gathers by summing the 4 partials per batch and adding c_proj_b.

Per-core kernel layout choices:
  - x^T [D, S] is staged on host so Q^T/K^T come out of matmuls directly with
    head_dim on partitions (what the scores matmul wants) and V comes out in
    [seq, head_dim] (what the AV matmul wants).
  - scores are computed transposed, sT[j, i] (j = key index on partitions), so
    the exp'd tile is directly usable as the AV matmul's moving operand.
  - softmax denominator comes from the SAME matmul as AV: each head's V
    window carries a ones column, [V|1] for the even head (numerator at psum
    partitions 0:64, rowsum at 64) and [1|gap63|V] for the odd head (rowsum
    at partition 0, numerator at 64:128; the gap multiplies into unread
    partitions).  Normalization is then fully lane-aligned: DVE reciprocals
    of the two rowsum rows, two 1-contraction broadcast matmuls into one
    psum tile (the odd one through a [0s|1s] selector row at partition 0,
    emitted first since its start= zeroes the bank height; matmul dst base
    must equal the lhsT tile position), one psum->SBUF stage (DVE reads only
    one PSUM operand), and two DVE muls straight into a^T.  This halves
    attention PE work vs a separate ones-matmul per probs tile and keeps
    every normalize off the DMA path.
  - QKV phase A runs k-outer while x^T/W stream from HBM: Q/K (i-blocks
    sc0-sc2) and V (j0-3) accumulate in all 8 PSUM banks, so PE tracks the
    input DMA instead of idling.  The rest of QKV + the output projection are
    emitted as keyed "filler" work units between attention steps (reserved
    for the stretches where the exp stream paces softmax), keeping PE busy
    end-to-end.  Diagonal j-tiles run first within each (quarter, pair) so
    the Pool tri-masks clear the drain's critical path.
  - psum->SBUF moves ride on ACT (Identity/Copy share exp's table set) where
    ACT has slack, on DVE otherwise.
  - x/Wqkv/scores/probs/V run in bf16; the output projection in float32r.
"""

from contextlib import ExitStack

import ml_dtypes
import numpy as np

B, S, D = 2, 2048, 1024
NH, HD = 16, 64
NCORES = 8
GROUPS = 4           # tensor-parallel head groups per batch
HPC = NH // GROUPS   # heads per core
SCALE = 1.0 / 8.0    # 1/sqrt(HD)
VBLK = 384           # per-j-tile V block: per pair [V_e|1|gap63|V_o]

_CACHE = {}


def _body(ctx, tc, mybir, xt, wqk, wv, wp, qkb, vb, tri, onesh, out):
    nc = tc.nc
    f32 = mybir.dt.float32
    f32r = mybir.dt.float32r
    bf16 = mybir.dt.bfloat16
    EXP = mybir.ActivationFunctionType.Exp

    pin = ctx.enter_context(tc.tile_pool(name="pin", bufs=1))
    pwork = ctx.enter_context(tc.tile_pool(name="pwork", bufs=1))
    ppt = ctx.enter_context(tc.tile_pool(name="ppt", bufs=12))
    prec = ctx.enter_context(tc.tile_pool(name="prec", bufs=6))
    pstage = ctx.enter_context(tc.tile_pool(name="pstage", bufs=6))
    ps_mm = ctx.enter_context(tc.tile_pool(name="ps_mm", bufs=2, space="PSUM"))
    ps_s = ctx.enter_context(tc.tile_pool(name="ps_s", bufs=2, space="PSUM"))
    ps_av = ctx.enter_context(tc.tile_pool(name="ps_av", bufs=1, space="PSUM"))

    # ---------------- input staging ----------------
    # Each DMA costs ~625ns of serial HWDGE time on top of its transfer, so
    # batch inputs into few, large DMAs ordered by first use: the first wqk
    # block + x^T chunk 0 unblock the first phase-A matmul ASAP, small
    # constants slot into the stream just before their first consumer.
    xt_sb = pin.tile([128, 8 * 2048], bf16, name="xt_sb")
    wqk_sb = pin.tile([128, 4096], bf16, name="wqk_sb")
    wv_sb = pin.tile([128, 2048], bf16, name="wv_sb")
    nc.sync.dma_start(wqk_sb[:, 0:512], wqk[:, 0:512])
    nc.sync.dma_start(xt_sb[:, 0:512], xt[0:128, 0:512])
    nc.sync.dma_start(wv_sb[:], wv[:])
    nc.sync.dma_start(xt_sb[:, 512:2048], xt[0:128, 512:2048])
    for k in range(1, 8):
        nc.sync.dma_start(wqk_sb[:, k * 512:(k + 1) * 512],
                            wqk[:, k * 512:(k + 1) * 512])
        nc.sync.dma_start(xt_sb[:, k * 2048:(k + 1) * 2048],
                          xt[k * 128:(k + 1) * 128, :])
    qkb_sb = pin.tile([128, 4], f32, name="qkb_sb")
    nc.sync.dma_start(qkb_sb[:], qkb[:])
    tri_sb = pin.tile([128, 128], bf16, name="tri_sb")
    nc.sync.dma_start(tri_sb[:], tri[:])
    vb_sb = pin.tile([128, 256], f32, name="vb_sb")
    nc.sync.dma_start(vb_sb[:], vb[:])
    ones64 = pin.tile([128, 128], bf16, name="ones64")
    nc.sync.dma_start(ones64[:], onesh[:])
    wp_sb = pin.tile([128, 2048], f32r, name="wp_sb")
    nc.sync.dma_start(wp_sb[:], wp[:])

    # Q^T / K^T: head-pair p at cols [p*2048, (p+1)*2048); head hh of the pair
    # on partitions [hh*64, hh*64+64).
    qt_sb = pwork.tile([128, 2 * 2048], bf16, name="qt_sb")
    kt_sb = pwork.tile([128, 2 * 2048], bf16, name="kt_sb")
    # V blocks of VBLK cols per j-tile: pair p at cols p*192, laid out
    # [V_even|1|gap63|V_odd] so the odd head's AV window [1|gap|V] puts its
    # rowsum at psum partition 0 and numerator at 64:128 (lane-aligned
    # normalize, no shift DMA); the gap multiplies into unread partitions.
    v_sb = pwork.tile([128, 16 * VBLK], bf16, name="v_sb")
    # a^T: k2 (head pair) at cols [k2*2048, ...), head hh on partitions hh*64..
    at_sb = pwork.tile([128, 2 * 2048], f32r, name="at_sb")
    # Dummy exp so the ACT table set loads during the input-DMA window instead
    # of delaying the first real softmax exp (wv lands within ~5us).
    warm = pin.tile([128, 4], f32, name="warm")
    nc.scalar.activation(warm[:], wv_sb[:, 0:4], EXP, scale=0.0)

    # ---------------- QKV helpers ----------------
    # col-tiles: C=0 -> Q pair0, C=1 -> Q pair1, C=2 -> K pair0, C=3 -> K pair1
    # psum->SBUF bias-add moves run on ACT (Copy is in the exp table set, and
    # ACT is idle during the QKV phases) to keep DVE free for softmax work.
    CPY = mybir.ActivationFunctionType.Copy
    IDN = mybir.ActivationFunctionType.Identity

    def qk_add(ps_ap, C, sc, on_act=True):
        dest = qt_sb if C < 2 else kt_sb
        p = C % 2
        dst = dest[:, p * 2048 + sc * 512: p * 2048 + (sc + 1) * 512]
        if on_act:
            nc.scalar.activation(dst, ps_ap, IDN, bias=qkb_sb[:, C:C + 1])
        else:  # late blocks run inside ACT-paced attention windows
            nc.vector.tensor_scalar_add(dst, ps_ap, qkb_sb[:, C:C + 1])

    def v_add(ps_ap256, j):
        blk = v_sb[:, j * VBLK:(j + 1) * VBLK].rearrange(
            "p (g c) -> p g c", c=192)
        src3 = ps_ap256.rearrange("p (g c) -> p g c", c=128)
        vb3 = vb_sb.rearrange("p (g c) -> p g c", c=128)
        nc.vector.tensor_add(blk[:, :, 0:64], src3[:, :, 0:64], vb3[:, :, 0:64])
        nc.vector.tensor_add(blk[:, :, 128:192], src3[:, :, 64:128],
                             vb3[:, :, 64:128])

    # ---------------- phase A: k-outer QKV subset ----------------
    # All 8 PSUM banks track the input stream: Q/K for sc0 (acc0/acc1),
    # sc1 (ava/avb), sc2 (packed 2-bank s tile), V j0-3 (packed s tile).
    a_q0 = ps_mm.tile([128, 512], f32, tag="acc", name="a_q0")
    a_k0 = ps_mm.tile([128, 512], f32, tag="acc", name="a_k0")
    a_q1 = ps_av.tile([128, 512], f32, tag="ava", name="a_q1")
    a_k1 = ps_av.tile([128, 512], f32, tag="avb", name="a_k1")
    a_v = ps_s.tile([128, 1024], f32, tag="s", name="a_v")
    a_qk2 = ps_s.tile([128, 1024], f32, tag="s", name="a_qk2")
    for k in range(8):
        for (ps_t, C, sc) in ((a_q0, 0, 0), (a_k0, 2, 0)):
            nc.tensor.matmul(
                ps_t[:],
                lhsT=wqk_sb[:, k * 512 + C * 128: k * 512 + (C + 1) * 128],
                rhs=xt_sb[:, k * 2048 + sc * 512: k * 2048 + (sc + 1) * 512],
                start=(k == 0), stop=(k == 7))
        for j in range(4):
            # start=True zeroes the whole 2KB bank region, so only the first
            # group per bank (j=0 for cols 0:512, j=2 for 512:1024) may start.
            nc.tensor.matmul(
                a_v[:, j * 256:(j + 1) * 256],
                lhsT=xt_sb[:, k * 2048 + j * 128: k * 2048 + (j + 1) * 128],
                rhs=wv_sb[:, k * 256:(k + 1) * 256],
                start=(k == 0 and j % 2 == 0), stop=(k == 7),
                skip_group_check=True)
        for (ps_t, C, sc) in ((a_q1, 0, 1), (a_k1, 2, 1)):
            nc.tensor.matmul(
                ps_t[:],
                lhsT=wqk_sb[:, k * 512 + C * 128: k * 512 + (C + 1) * 128],
                rhs=xt_sb[:, k * 2048 + sc * 512: k * 2048 + (sc + 1) * 512],
                start=(k == 0), stop=(k == 7))
        for (co, C) in ((0, 0), (512, 2)):  # sc2, one group per bank
            nc.tensor.matmul(
                a_qk2[:, co:co + 512],
                lhsT=wqk_sb[:, k * 512 + C * 128: k * 512 + (C + 1) * 128],
                rhs=xt_sb[:, k * 2048 + 1024: k * 2048 + 1536],
                start=(k == 0), stop=(k == 7))
    qk_add(a_k0[:], 2, 0)
    qk_add(a_q0[:], 0, 0)
    for j in range(4):
        v_add(a_v[:, j * 256:(j + 1) * 256], j)
    qk_add(a_q1[:], 0, 1)
    qk_add(a_k1[:], 2, 1)
    qk_add(a_qk2[:, 0:512], 0, 2)
    qk_add(a_qk2[:, 512:1024], 2, 2)
    # ones columns of the V blocks (emitted after the v_adds so the DVE queue
    # is not head-blocked waiting for the onesh DMA; disjoint columns)
    nc.vector.tensor_copy(
        v_sb.rearrange("p (g c) -> p g c", c=192)[:, :, 64:65],
        ones64[:, 64:128].rearrange("p (g c) -> p g c", c=2)[:, :, 0:1])
    # the odd-head AV window spans gap cols 65:128 of each 192-group; they
    # multiply into unread psum partitions 1:64, but must be initialized
    # (and finite) -- fill them with arbitrary resident data
    nc.vector.tensor_copy(
        v_sb.rearrange("p (g c) -> p g c", c=192)[:, :, 65:128],
        xt_sb[:, 0:32 * 63].rearrange("p (g c) -> p g c", c=63))

    # ---------------- filler work units ----------------
    def qk_block(sc, C, on_act=True):
        def go():
            ps = ps_mm.tile([128, 512], f32, tag="acc", name="qkB")
            for k in range(8):
                nc.tensor.matmul(
                    ps[:],
                    lhsT=wqk_sb[:, k * 512 + C * 128: k * 512 + (C + 1) * 128],
                    rhs=xt_sb[:, k * 2048 + sc * 512: k * 2048 + (sc + 1) * 512],
                    start=(k == 0), stop=(k == 7))
            qk_add(ps[:], C, sc, on_act=on_act)
        return go

    def v_block(j):
        def go():
            ps = ps_mm.tile([128, 256], f32, tag="acc", name="vB")
            for k in range(8):
                nc.tensor.matmul(
                    ps[:],
                    lhsT=xt_sb[:, k * 2048 + j * 128: k * 2048 + (j + 1) * 128],
                    rhs=wv_sb[:, k * 256:(k + 1) * 256],
                    start=(k == 0), stop=(k == 7))
            v_add(ps[:], j)
        return go

    def proj_stile(st, on_act=False):
        def go():
            stage = pstage.tile([128, 1024], bf16, tag="stage", name="stage")
            for ec in range(2):
                ps = ps_mm.tile([128, 512], f32, tag="acc", name="ps_o")
                for k2 in range(2):
                    nc.tensor.matmul(
                        ps[:],
                        lhsT=at_sb[:, k2 * 2048 + st * 128: k2 * 2048 + (st + 1) * 128],
                        rhs=wp_sb[:, k2 * 1024 + ec * 512: k2 * 1024 + (ec + 1) * 512],
                        start=(k2 == 0), stop=(k2 == 1))
                if on_act:  # tail stiles: ACT is idle once the exps are done
                    nc.scalar.activation(stage[:, ec * 512:(ec + 1) * 512],
                                         ps[:], CPY)
                else:
                    nc.vector.tensor_copy(stage[:, ec * 512:(ec + 1) * 512],
                                          ps[:])
            nc.sync.dma_start(out[st * 128:(st + 1) * 128, :], stage[:])
        return go

    # Filler queue: (key, closure) where key = 2*Q + p of the earliest
    # attention instance allowed to pop it.  Reserving late work for the
    # ACT-saturated Q2/Q3 stretches keeps PE fed end-to-end.
    filler = [(0, qk_block(0, 1)), (0, qk_block(0, 3)),
              (0, qk_block(1, 1)), (0, qk_block(1, 3)),
              (1, v_block(4)), (1, v_block(5)),
              (1, v_block(6)), (1, v_block(7)),
              (2, qk_block(3, 0, False)), (2, qk_block(3, 2, False)),
              (2, v_block(8)), (2, v_block(9)),
              (3, v_block(10)), (3, v_block(11)),
              (4, qk_block(2, 1, False)), (4, qk_block(2, 3, False)),
              (6, qk_block(3, 1, False)), (6, qk_block(3, 3, False)),
              (6, v_block(12)), (6, v_block(13)),
              (6, v_block(14)), (6, v_block(15))]

    def pop_filler(key):
        for i, (mk, go) in enumerate(filler):
            if mk <= key:
                filler.pop(i)
                go()
                return

    pend = []  # deferred attention finishers (normalize + a^T write)

    # ---------------- attention ----------------
    # Per (i-quarter Q of 512, head-pair p).  Scores for both heads of the
    # pair share one [128, 1024] PSUM tile (head hh at cols hh*512) so one
    # strided exp covers both.  AV psums (per head):
    #   av[0:65] = [V|1]^T probs  -> rows 0:64 numerator, row 64 rowsum
    # The finisher normalizes lane-aligned at partitions 0:64 and lane-shifts
    # the odd head's a^T to partitions 64:128 with a SBUF->SBUF DMA.
    DELAY = 6  # software-pipeline distance between scores/exp and AV use

    def att_qp(Q, p):
        qlo = Q * 512
        Jmax = 4 * Q + 3
        nJ = 4 * Q + 4
        ava = ps_av.tile([128, 512], f32, tag="ava", name="ava")
        avb = ps_av.tile([128, 512], f32, tag="avb", name="avb")
        pts = []
        # Diagonal j-tiles first: PSUM accumulation is commutative, and this
        # moves the Pool tri-masks off the drain-phase critical path (the
        # closing AVs then consume mask-free full tiles).
        jorder = list(range(4 * Q, nJ)) + list(range(0, 4 * Q))
        for idx in range(nJ + DELAY):
            if idx < nJ:
                jlo = jorder[idx] * 128
                istart = max(jlo, qlo)
                w = qlo + 512 - istart
                pss = ps_s.tile([128, 1024], f32, tag="s", name="pss")
                for hh in range(2):
                    nc.tensor.matmul(
                        pss[:, hh * 512: hh * 512 + w],
                        lhsT=kt_sb[hh * 64:(hh + 1) * 64, p * 2048 + jlo: p * 2048 + jlo + 128],
                        rhs=qt_sb[hh * 64:(hh + 1) * 64, p * 2048 + istart: p * 2048 + istart + w],
                        start=True, stop=True)
                pt = ppt.tile([128, 1024], bf16, tag="pt", name="pt")
                nc.scalar.activation(
                    pt.rearrange("x (h c) -> x h c", c=512)[:, :, 0:w],
                    pss.rearrange("x (h c) -> x h c", c=512)[:, :, 0:w],
                    EXP, scale=SCALE)
                if jlo >= qlo:
                    # diagonal j-tile: zero the j > i triangle
                    nc.gpsimd.tensor_mul(pt[:, 0:128], pt[:, 0:128], tri_sb[:])
                    nc.gpsimd.tensor_mul(pt[:, 512:640], pt[:, 512:640], tri_sb[:])
                pts.append((pt, istart - qlo, w))
            if idx == 1 and pend:
                pend.pop(0)()
            # Q3: force the urgent pops early (v12-15 must beat the first
            # AVs), then hold the rest for the ACT-bound drain steps.
            if (Q, p) == (3, 0):
                if idx < 6 or idx >= 16:
                    pop_filler(6)
            elif (Q, p) == (3, 1):
                if idx >= 15:
                    pop_filler(7)
            elif idx % 2 == 0:
                pop_filler(2 * Q + p)
            ia = idx - DELAY
            if ia < 0:
                continue
            pt, co, w = pts[ia]
            base = jorder[ia] * VBLK + p * 192
            kw = dict(start=(ia == 0), stop=(ia == Jmax), skip_group_check=True)
            nc.tensor.matmul(ava[0:65, co:512], lhsT=v_sb[:, base: base + 65],
                             rhs=pt[:, 0:w], **kw)
            nc.tensor.matmul(avb[0:128, co:512], lhsT=v_sb[:, base + 64: base + 192],
                             rhs=pt[:, 512:512 + w], **kw)

        if (Q, p) != (3, 1):
            pop_filler(2 * Q + p)

        def finish():
            rec = prec.tile([128, 512], bf16, tag="rec", name="rec")
            with nc.allow_low_precision(reason="softmax denominators in bf16"):
                nc.vector.reciprocal(rec[64:65, :], ava[64:65, :])
                nc.vector.reciprocal(rec[0:1, :], avb[0:1, :])
            # broadcast the reciprocal rows over each head's partitions; DVE
            # may read only one PSUM operand, so stage the broadcast in SBUF.
            # Matmul dst base must equal lhsT tile position, so the odd head
            # broadcasts through a [0s|1s] selector row at partition 0 (out
            # base 0, values land at 64:128) -- emitted first, since its
            # start= zeroes the whole bank height.
            bc = ps_mm.tile([128, 512], f32, tag="acc", name="bc")
            nc.tensor.matmul(bc[0:128, :], lhsT=ones64[0:1, 0:128],
                             rhs=rec[0:1, :], start=True, stop=True,
                             skip_group_check=True)
            nc.tensor.matmul(bc[0:64, :], lhsT=ones64[64:65, 64:128],
                             rhs=rec[64:65, :], start=True, stop=True,
                             skip_group_check=True)
            bc_sb = prec.tile([128, 512], f32r, tag="bcs", name="bc_sb")
            if Q < 2:  # ACT has slack in the early quarters
                nc.scalar.activation(bc_sb[:], bc[:], CPY)
            else:
                nc.vector.tensor_copy(bc_sb[:], bc[:])
            nc.vector.tensor_mul(
                at_sb[0:64, p * 2048 + qlo: p * 2048 + qlo + 512],
                ava[0:64, :], bc_sb[0:64, :])
            nc.vector.tensor_mul(
                at_sb[64:128, p * 2048 + qlo: p * 2048 + qlo + 512],
                avb[64:128, :], bc_sb[64:128, :])
            if p == 1 and Q < 3:
                # Q's proj stiles: spread over the later, ACT-bound stretches
                keys = {0: (2, 2, 2, 2), 1: (4, 4, 5, 5), 2: (6, 6, 7, 7)}[Q]
                filler.extend(
                    (kk, proj_stile(st))
                    for kk, st in zip(keys, range(4 * Q, 4 * Q + 4)))
        pend.append(finish)

    for Q in range(4):
        att_qp(Q, 0)
        att_qp(Q, 1)
    # tail: overlap st12's pair-0 accumulation (PSUM held open in the freed
    # score slots) with the last finisher's normalize chain, then finish the
    # remaining stiles with ACT copies while DVE/DMA drain.
    split = []
    for ec in range(2):
        ps = ps_s.tile([128, 512], f32, tag="s", name="ps_f")
        nc.tensor.matmul(ps[:], lhsT=at_sb[:, 12 * 128: 13 * 128],
                         rhs=wp_sb[:, ec * 512:(ec + 1) * 512],
                         start=True, stop=False)
        split.append(ps)
    while pend:
        pend.pop(0)()
    while filler:
        # leftover stiles overlap the last finisher's DVE normalize chain
        filler.pop(0)[1]()
    stage12 = pstage.tile([128, 1024], bf16, tag="stage", name="stage12")
    for ec in range(2):
        nc.tensor.matmul(split[ec][:],
                         lhsT=at_sb[:, 2048 + 12 * 128: 2048 + 13 * 128],
                         rhs=wp_sb[:, 1024 + ec * 512: 1024 + (ec + 1) * 512],
                         start=False, stop=True)
        if ec == 0:
            nc.scalar.activation(stage12[:, 0:512], split[ec][:], CPY)
        else:
            nc.vector.tensor_copy(stage12[:, 512:1024], split[ec][:])
    nc.sync.dma_start(out[12 * 128:13 * 128, :], stage12[:])
    for st in (13, 14, 15):
        stage = pstage.tile([128, 1024], bf16, tag="stage", name="stageT")
        for ec in range(2):
            ps = ps_mm.tile([128, 512], f32, tag="acc", name="ps_t")
            for k2 in range(2):
                nc.tensor.matmul(
                    ps[:],
                    lhsT=at_sb[:, k2 * 2048 + st * 128: k2 * 2048 + (st + 1) * 128],
                    rhs=wp_sb[:, k2 * 1024 + ec * 512: k2 * 1024 + (ec + 1) * 512],
                    start=(k2 == 0), stop=(k2 == 1))
            if ec == 0:
                nc.scalar.activation(stage[:, 0:512], ps[:], CPY)
            else:
                nc.vector.tensor_copy(stage[:, 512:1024], ps[:])
            if st >= 14:  # per-half DMAs shorten the closing drain
                nc.sync.dma_start(
                    out[st * 128:(st + 1) * 128, ec * 512:(ec + 1) * 512],
                    stage[:, ec * 512:(ec + 1) * 512])
        if st < 14:
            nc.sync.dma_start(out[st * 128:(st + 1) * 128, :], stage[:])


def _build_nc(repeat=1):
    key = ("nc", repeat)
    if key in _CACHE:
        return _CACHE[key]
    import concourse.bacc as bacc
    import concourse.mybir as mybir
    import concourse.tile as tile

    f32 = mybir.dt.float32
    f32r = mybir.dt.float32r
    bf16d = mybir.dt.bfloat16
    nc = bacc.Bacc("TRN2", target_bir_lowering=False, debug=False)
    xt = nc.dram_tensor("xt", [D, S], bf16d, kind="ExternalInput").ap()
    wqk = nc.dram_tensor("wqk", [128, 4096], bf16d, kind="ExternalInput").ap()
    wv = nc.dram_tensor("wv", [128, 2048], bf16d, kind="ExternalInput").ap()
    wp = nc.dram_tensor("wp", [128, 2048], f32r, kind="ExternalInput").ap()
    qkb = nc.dram_tensor("qkb", [128, 4], f32, kind="ExternalInput").ap()
    vb = nc.dram_tensor("vb", [128, 256], f32, kind="ExternalInput").ap()
    tri = nc.dram_tensor("tri", [128, 128], bf16d, kind="ExternalInput").ap()
    onesh = nc.dram_tensor("onesh", [128, 128], bf16d, kind="ExternalInput").ap()
    out = nc.dram_tensor("out", [S, D], bf16d, kind="ExternalOutput").ap()

    with tile.TileContext(nc) as tc:
        for _ in range(repeat):
            with ExitStack() as ctx:
                _body(ctx, tc, mybir, xt, wqk, wv, wp, qkb, vb, tri, onesh, out)
    nc.compile()
    _CACHE[key] = nc
    return nc


def _make_in_maps(hidden_states, c_attn_w, c_attn_b, c_proj_w):
    hs = np.asarray(hidden_states, dtype=np.float32)
    waw = np.asarray(c_attn_w, dtype=np.float32)
    wab = np.asarray(c_attn_b, dtype=np.float32)
    wpw = np.asarray(c_proj_w, dtype=np.float32)

    tri = np.triu(np.ones((128, 128), dtype=ml_dtypes.bfloat16))
    xts = [np.ascontiguousarray(hs[b].T).astype(ml_dtypes.bfloat16) for b in range(B)]
    in_maps = []
    for c in range(NCORES):
        b, g = divmod(c, GROUPS)
        cols = np.arange(g * HPC * HD, (g + 1) * HPC * HD)
        wqk_host = np.concatenate([waw[:, cols], waw[:, D + cols]], axis=1)
        in_maps.append({
            "xt": xts[b],
            "wqk": np.ascontiguousarray(
                wqk_host.reshape(8, 128, 512).transpose(1, 0, 2).reshape(128, 4096)).astype(ml_dtypes.bfloat16),
            "wv": np.ascontiguousarray(
                waw[:, 2 * D + cols].reshape(8, 128, 256).transpose(1, 0, 2).reshape(128, 2048)).astype(ml_dtypes.bfloat16),
            "wp": np.ascontiguousarray(
                wpw[cols, :].reshape(2, 128, 1024).transpose(1, 0, 2).reshape(128, 2048)),
            "qkb": np.ascontiguousarray(
                np.concatenate([wab[cols], wab[D + cols]]).reshape(4, 128).T),
            "vb": np.ascontiguousarray(
                np.broadcast_to(wab[2 * D + cols], (128, 256))),
            "tri": tri,
            "onesh": np.concatenate(
                [np.zeros((128, 64), np.float32), np.ones((128, 64), np.float32)],
                axis=1).astype(ml_dtypes.bfloat16),
        })
    return in_maps


def kernel(hidden_states, c_attn_w, c_attn_b, c_proj_w, c_proj_b):
    from concourse import bass_utils

    nc = _build_nc()
    in_maps = _make_in_maps(hidden_states, c_attn_w, c_attn_b, c_proj_w)
    res = bass_utils.run_bass_kernel_spmd(nc, in_maps, core_ids=list(range(NCORES)))
    outs = [np.asarray(r["out"], dtype=np.float32) for r in res.results]
    wpb = np.asarray(c_proj_b, dtype=np.float32)
    full = np.stack(
        [sum(outs[b * GROUPS:(b + 1) * GROUPS]) + wpb for b in range(B)], axis=0)
    return full.astype(np.float32)
